# revision 7
# baseline (speedup 1.0000x reference)
"""GAE (Generalized Advantage Estimation) Bass kernel for 8 Trainium2 cores.

Problem: rewards (2048, 8192) f32, values (2048, 8192) f32,
next_values (2048,) f32.
  next_v[:, t] = values[:, t+1] (t < S-1), next_values (t = S-1)
  deltas = rewards + GAMMA * next_v - values
  A_t = deltas_t + (GAMMA*LAM) * A_{t+1}   (A_S = 0, backward recurrence)
  advantages = A, returns = A + values

Sharding: pure data parallel over the batch dim — 2048 rows / 8 cores =
256 rows per core; the seq recurrence is row-local so there is no
cross-core communication.

Per core: 2 partition tiles of 128 rows x 8192 seq. The seq dim is cut
into chunks; each chunk's backward recurrence runs as a single DVE
tensor_tensor_scan over a reversed (negative-stride) view, chained
right-to-left across chunks through the scan's `initial` operand.
Everything stays in the natural [batch, seq] layout, so all DMAs move
long contiguous runs and the kernel is HBM-bound (~32MB/core traffic).
"""

import sys

if "/opt/trn_rl_repo" not in sys.path:
    sys.path.insert(0, "/opt/trn_rl_repo")

import numpy as np

GAMMA = 0.99
LAM = 0.95
C_COEF = GAMMA * LAM

B, S = 2048, 8192
N_CORES = 8
ROWS = B // N_CORES  # 256 rows per core
P = 128  # SBUF partitions
N_TILES = ROWS // P  # 2 row-tiles per core
# Chunk widths, right-to-left. Small edge chunks shorten pipeline fill
# (first scan starts after only a 256KB load) and drain (last store is
# small); middle chunks are large to amortize per-instruction overhead.
CHUNK_MAX = 2048
CHUNKS_R2L = [512, 1536, 2048, 2048, 1536, 512]
assert sum(CHUNKS_R2L) == S

_CACHE: dict = {}


def _build():
    import concourse.bacc as bacc
    import concourse.mybir as mybir
    from concourse.tile import TileContext

    f32 = mybir.dt.float32
    add = mybir.AluOpType.add
    sub = mybir.AluOpType.subtract
    mult = mybir.AluOpType.mult

    nc = bacc.Bacc("TRN2", target_bir_lowering=False, name="gae8")
    r = nc.dram_tensor("rewards", [ROWS, S], f32, kind="ExternalInput")
    v = nc.dram_tensor("values", [ROWS, S], f32, kind="ExternalInput")
    nv = nc.dram_tensor("next_values", [ROWS], f32, kind="ExternalInput")
    adv = nc.dram_tensor("adv", [ROWS, S], f32, kind="ExternalOutput")
    ret = nc.dram_tensor("ret", [ROWS, S], f32, kind="ExternalOutput")

    # Returns satisfy their own backward recurrence, which needs one fewer
    # elementwise pass than the advantages form:
    #   B_t = e_t + c*B_{t+1},  e_t = r_t + gamma*(1-lam)*v_{t+1},  B_S = nv
    #   returns = B, advantages = B - v
    g1ml = GAMMA * (1.0 - LAM)

    with TileContext(nc) as tc:
        with (
            tc.tile_pool(name="cpool", bufs=1) as cpool,
            tc.tile_pool(name="pool", bufs=4) as pool,
        ):
            c_t = cpool.tile([P, 1], f32)
            nc.vector.memset(c_t[:, :], C_COEF)

            # The two 128-row tiles are independent recurrence chains;
            # interleave them chunk-by-chunk so neither chain cold-starts
            # mid-kernel and the pipeline tail is two small chunks.
            prev_ret = [None] * N_TILES
            prev_v = [None] * N_TILES
            col_end = [S] * N_TILES
            for i, w in enumerate(CHUNKS_R2L):
                for t in range(N_TILES):
                    rows = slice(t * P, (t + 1) * P)
                    col = col_end[t] - w
                    first = i == 0
                    v_t = pool.tile([P, CHUNK_MAX + 1], f32)
                    r_t = pool.tile([P, CHUNK_MAX], f32)
                    ret_t = pool.tile([P, CHUNK_MAX], f32)
                    adv_t = pool.tile([P, CHUNK_MAX], f32)

                    # Aligned loads only (odd widths split DMA packets into
                    # 4B stragglers that choke the ring).
                    nc.sync.dma_start(out=v_t[:, 0:w], in_=v[rows, col : col + w])
                    # The very first chunks' r-loads ride the store ring
                    # (idle during fill) so the edge loads run in parallel.
                    rdma = nc.scalar if first else nc.sync
                    rdma.dma_start(out=r_t[:, 0:w], in_=r[rows, col : col + w])
                    # Successor column v[col+w]: next_values at the right
                    # edge (tiny SWDGE DMA, off the main rings), else first
                    # column of the chunk to the right, already on-chip
                    # (tiny ACT copy off the DVE).
                    if first:
                        nc.gpsimd.dma_start(
                            out=v_t[:, w : w + 1],
                            in_=nv[t * P : (t + 1) * P].unsqueeze(1),
                        )
                    else:
                        nc.scalar.copy(
                            out=v_t[:, w : w + 1], in_=prev_v[t][:, 0:1]
                        )

                    # e = g1ml * v_next + r  (in place over r_t)
                    nc.vector.scalar_tensor_tensor(
                        out=r_t[:, 0:w],
                        in0=v_t[:, 1 : w + 1],
                        scalar=g1ml,
                        in1=r_t[:, 0:w],
                        op0=mult,
                        op1=add,
                    )
                    # backward recurrence over reversed views:
                    # state = c*state + e -> returns; carry chains via initial.
                    # Rightmost chunk: initial = next_values (v_t's extra col).
                    init = (
                        v_t[:, w : w + 1]
                        if prev_ret[t] is None
                        else prev_ret[t][:, 0:1]
                    )
                    nc.vector.tensor_tensor_scan(
                        out=ret_t[:, 0:w][:, ::-1],
                        data0=c_t[:, :].broadcast_to([P, w]),
                        data1=r_t[:, 0:w][:, ::-1],
                        initial=init,
                        op0=mult,
                        op1=add,
                    )
                    # advantages = returns - v (DVE: GpSimd contends for SBUF
                    # ports with DVE and halves scan throughput)
                    nc.vector.tensor_tensor(
                        out=adv_t[:, 0:w],
                        in0=ret_t[:, 0:w],
                        in1=v_t[:, 0:w],
                        op=sub,
                    )

                    # stores go out the scalar-engine HWDGE ring
                    # (qActDynamicHW) so they don't FIFO-serialize behind
                    # later chunks' loads on the sync ring (qSPDynamicHW).
                    nc.scalar.dma_start(
                        out=ret[rows, col : col + w], in_=ret_t[:, 0:w]
                    )
                    nc.scalar.dma_start(
                        out=adv[rows, col : col + w], in_=adv_t[:, 0:w]
                    )
                    prev_ret[t] = ret_t
                    prev_v[t] = v_t
                    col_end[t] = col
    nc.finalize()
    return nc


def _get_nc():
    if "nc" not in _CACHE:
        _CACHE["nc"] = _build()
    return _CACHE["nc"]


def _run(rewards, values, next_values, **spmd_kwargs):
    """Shard over cores, run the Bass kernel, return BassKernelResults."""
    from concourse.bass_utils import run_bass_kernel_spmd

    nc = _get_nc()
    in_maps = []
    for c in range(N_CORES):
        sl = slice(c * ROWS, (c + 1) * ROWS)
        in_maps.append(
            {
                "rewards": np.ascontiguousarray(rewards[sl], dtype=np.float32),
                "values": np.ascontiguousarray(values[sl], dtype=np.float32),
                "next_values": np.ascontiguousarray(
                    next_values[sl], dtype=np.float32
                ),
            }
        )
    return run_bass_kernel_spmd(
        nc, in_maps, core_ids=list(range(N_CORES)), **spmd_kwargs
    )


def kernel(rewards, values, next_values):
    res = _run(rewards, values, next_values)
    advantages = np.concatenate([res.results[c]["adv"] for c in range(N_CORES)], 0)
    returns = np.concatenate([res.results[c]["ret"] for c in range(N_CORES)], 0)
    return advantages, returns


# revision 9
# speedup vs baseline: 1.0491x; 1.0491x over previous
"""GAE (Generalized Advantage Estimation) Bass kernel for 8 Trainium2 cores.

Problem: rewards (2048, 8192) f32, values (2048, 8192) f32,
next_values (2048,) f32.
  next_v[:, t] = values[:, t+1] (t < S-1), next_values (t = S-1)
  deltas = rewards + GAMMA * next_v - values
  A_t = deltas_t + (GAMMA*LAM) * A_{t+1}   (A_S = 0, backward recurrence)
  advantages = A, returns = A + values

Sharding: pure data parallel over the batch dim — 2048 rows / 8 cores =
256 rows per core; the seq recurrence is row-local so there is no
cross-core communication.

Per core: 2 partition tiles of 128 rows x 8192 seq. The seq dim is cut
into chunks; each chunk's backward recurrence runs as a single DVE
tensor_tensor_scan over a reversed (negative-stride) view, chained
right-to-left across chunks through the scan's `initial` operand.
Everything stays in the natural [batch, seq] layout, so all DMAs move
long contiguous runs and the kernel is HBM-bound (~32MB/core traffic).
"""

import sys

if "/opt/trn_rl_repo" not in sys.path:
    sys.path.insert(0, "/opt/trn_rl_repo")

import numpy as np

GAMMA = 0.99
LAM = 0.95
C_COEF = GAMMA * LAM

B, S = 2048, 8192
N_CORES = 8
ROWS = B // N_CORES  # 256 rows per core
P = 128  # SBUF partitions
N_TILES = ROWS // P  # 2 row-tiles per core
# DMA granularity: every load/store moves a [128, 2048] pair-tile (8KB
# per partition line) to amortize per-instruction ring overhead. Compute
# sub-chunks inside each pair ramp down at the edges so the first scan
# starts early and the last scan is short.  Lists are right-to-left.
PAIR = 2048
N_PAIRS = S // PAIR
# per-pair compute sub-chunk widths, right-to-left within the pair
PAIR_SUBS = {0: [512, 1536], N_PAIRS - 1: [1536, 512]}

_CACHE: dict = {}


def _build():
    import concourse.bacc as bacc
    import concourse.mybir as mybir
    from concourse.tile import TileContext

    f32 = mybir.dt.float32
    add = mybir.AluOpType.add
    sub = mybir.AluOpType.subtract
    mult = mybir.AluOpType.mult

    nc = bacc.Bacc("TRN2", target_bir_lowering=False, name="gae8")
    r = nc.dram_tensor("rewards", [ROWS, S], f32, kind="ExternalInput")
    v = nc.dram_tensor("values", [ROWS, S], f32, kind="ExternalInput")
    nv = nc.dram_tensor("next_values", [ROWS], f32, kind="ExternalInput")
    adv = nc.dram_tensor("adv", [ROWS, S], f32, kind="ExternalOutput")
    ret = nc.dram_tensor("ret", [ROWS, S], f32, kind="ExternalOutput")

    # Returns satisfy their own backward recurrence, which needs one fewer
    # elementwise pass than the advantages form:
    #   B_t = e_t + c*B_{t+1},  e_t = r_t + gamma*(1-lam)*v_{t+1},  B_S = nv
    #   returns = B, advantages = B - v
    g1ml = GAMMA * (1.0 - LAM)

    with TileContext(nc) as tc:
        with (
            tc.tile_pool(name="cpool", bufs=1) as cpool,
            tc.tile_pool(name="boot", bufs=1) as boot,
            tc.tile_pool(name="pool", bufs=6) as pool,
        ):
            c_t = cpool.tile([P, 1], f32)
            nc.vector.memset(c_t[:, :], C_COEF)

            # Hoist the second row-tile's first-pair loads to program start
            # so its chain never stalls on the load ring mid-kernel. Its
            # r-load (and tile 0's) ride the store ring, idle during fill.
            boot_v = boot.tile([P, PAIR + 1], f32)
            boot_r = boot.tile([P, PAIR], f32)
            rows1 = slice(P, 2 * P)
            nc.sync.dma_start(
                out=boot_v[:, 0:PAIR], in_=v[rows1, S - PAIR : S]
            )
            nc.scalar.dma_start(
                out=boot_r[:, :], in_=r[rows1, S - PAIR : S]
            )
            nc.gpsimd.dma_start(
                out=boot_v[:, PAIR : PAIR + 1],
                in_=nv[P : 2 * P].unsqueeze(1),
            )

            for t in range(N_TILES):
                rows = slice(t * P, (t + 1) * P)
                prev_ret = None
                prev_v = None
                # pairs processed right-to-left
                for pi in range(N_PAIRS - 1, -1, -1):
                    # right-to-left pair index (0 = rightmost)
                    pr = N_PAIRS - 1 - pi
                    col0 = pi * PAIR
                    if t == 1 and pr == 0:
                        v_t, r_t = boot_v, boot_r
                    else:
                        v_t = pool.tile([P, PAIR + 1], f32)
                        r_t = pool.tile([P, PAIR], f32)
                        nc.sync.dma_start(
                            out=v_t[:, 0:PAIR], in_=v[rows, col0 : col0 + PAIR]
                        )
                        rdma = nc.scalar if pr == 0 else nc.sync
                        rdma.dma_start(
                            out=r_t[:, :], in_=r[rows, col0 : col0 + PAIR]
                        )
                        # Successor column v[col0+PAIR]: next_values at the
                        # right edge (tiny SWDGE DMA off the main rings),
                        # else col 0 of the pair to the right, already
                        # on-chip (tiny ACT copy off the DVE).
                        if pr == 0:
                            nc.gpsimd.dma_start(
                                out=v_t[:, PAIR : PAIR + 1],
                                in_=nv[t * P : (t + 1) * P].unsqueeze(1),
                            )
                        else:
                            nc.scalar.copy(
                                out=v_t[:, PAIR : PAIR + 1], in_=prev_v[:, 0:1]
                            )
                    ret_t = pool.tile([P, PAIR], f32)

                    # compute sub-chunks (right-to-left inside the pair)
                    b = PAIR
                    for w in PAIR_SUBS.get(pr, [PAIR]):
                        a = b - w
                        # e = g1ml * v_next + r  (in place over r_t)
                        nc.vector.scalar_tensor_tensor(
                            out=r_t[:, a:b],
                            in0=v_t[:, a + 1 : b + 1],
                            scalar=g1ml,
                            in1=r_t[:, a:b],
                            op0=mult,
                            op1=add,
                        )
                        # backward recurrence over reversed views:
                        # state = c*state + e -> returns; carry chains via
                        # initial: next_values at the global right edge,
                        # else the previously computed column to the right.
                        if b == PAIR:
                            init = (
                                v_t[:, PAIR : PAIR + 1]
                                if prev_ret is None
                                else prev_ret[:, 0:1]
                            )
                        else:
                            init = ret_t[:, b : b + 1]
                        nc.vector.tensor_tensor_scan(
                            out=ret_t[:, a:b][:, ::-1],
                            data0=c_t[:, :].broadcast_to([P, w]),
                            data1=r_t[:, a:b][:, ::-1],
                            initial=init,
                            op0=mult,
                            op1=add,
                        )
                        # advantages = returns - v, written into the freed
                        # e slots (DVE: GpSimd contends for SBUF ports with
                        # DVE and halves scan throughput)
                        nc.vector.tensor_tensor(
                            out=r_t[:, a:b],
                            in0=ret_t[:, a:b],
                            in1=v_t[:, a:b],
                            op=sub,
                        )
                        b = a

                    # pair-level stores on the scalar-engine HWDGE ring
                    # (qActDynamicHW) so they don't FIFO-serialize behind
                    # later loads on the sync ring (qSPDynamicHW).
                    nc.scalar.dma_start(
                        out=ret[rows, col0 : col0 + PAIR], in_=ret_t[:, :]
                    )
                    nc.scalar.dma_start(
                        out=adv[rows, col0 : col0 + PAIR], in_=r_t[:, :]
                    )
                    prev_ret = ret_t
                    prev_v = v_t
    nc.finalize()
    return nc


def _get_nc():
    if "nc" not in _CACHE:
        _CACHE["nc"] = _build()
    return _CACHE["nc"]


def _run(rewards, values, next_values, **spmd_kwargs):
    """Shard over cores, run the Bass kernel, return BassKernelResults."""
    from concourse.bass_utils import run_bass_kernel_spmd

    nc = _get_nc()
    in_maps = []
    for c in range(N_CORES):
        sl = slice(c * ROWS, (c + 1) * ROWS)
        in_maps.append(
            {
                "rewards": np.ascontiguousarray(rewards[sl], dtype=np.float32),
                "values": np.ascontiguousarray(values[sl], dtype=np.float32),
                "next_values": np.ascontiguousarray(
                    next_values[sl], dtype=np.float32
                ),
            }
        )
    return run_bass_kernel_spmd(
        nc, in_maps, core_ids=list(range(N_CORES)), **spmd_kwargs
    )


def kernel(rewards, values, next_values):
    res = _run(rewards, values, next_values)
    advantages = np.concatenate([res.results[c]["adv"] for c in range(N_CORES)], 0)
    returns = np.concatenate([res.results[c]["ret"] for c in range(N_CORES)], 0)
    return advantages, returns
